# revision 6
# baseline (speedup 1.0000x reference)
"""DiT block with MoE — Trainium2 Bass/Tile kernel, 8-core SPMD.

Sharding: tokens split across 8 cores (cores 0-3 batch 0, cores 4-7 batch 1,
512 tokens each).  Activations are kept dim-major ([DIM, tokens]) on chip so
every projection / attention step is a dense chain of 128x128x512 bf16
matmuls with no on-device transposes:
  - layernorm / rmsnorm partition reductions -> ones-vector matmuls
  - per-token broadcasts -> K=1 ones matmuls
  - RoPE pair rotation -> host-side even/odd permutation of W_q/W_k output
    columns (scores are invariant to a shared per-head permutation of q,k)
  - softmax runs max-free (scores are O(5) for rmsnorm'd q,k), denominators
    via ones-matmuls
  - MoE top-k combine -> host-computed per-token expert mask rows; each
    expert output is (psum + b_e) * mask_e in one fused DVE op, accumulated
Self/cross-attention K,V shards are exchanged with one AllGather inside each
4-core batch group.  All matmuls bf16 with fp32 PSUM accumulation; the
residual stream stays fp32 (streamed through DRAM scratch between stages).
"""

import contextlib
import os

import numpy as np
import ml_dtypes

import concourse.bass as bass
import concourse.tile as tile
import concourse.mybir as mybir
from concourse import bacc, bass_utils
from concourse.bass import ds, ts

B, S, CTX, DIM, NH, FFN, NE, TOPK = 2, 2048, 512, 1536, 12, 6144, 4, 2
HD = DIM // NH          # 128
EPS = 1e-6
N_CORES = 8
CPB = N_CORES // B      # cores per batch = 4
TPC = S // CPB          # tokens per core = 512
CTXC = CTX // CPB       # ctx tokens per core = 128
KC = DIM // 128         # dim chunks = 12
FC = FFN // 128         # ffn chunks = 48
HT = TPC // 2           # token half for ffn/moe stage = 256
BF16 = mybir.dt.bfloat16
F32 = mybir.dt.float32
bfnp = ml_dtypes.bfloat16

ACTF = mybir.ActivationFunctionType
ALU = mybir.AluOpType

SZ_K = DIM * TPC
SZ_V = TPC * DIM
SZ_CK = DIM * CTXC
SZ_CV = CTXC * DIM
GATHER_ELEMS = SZ_K + SZ_V + SZ_CK + SZ_CV

_VEC_NAMES = [
    "s1p", "sh1", "g1", "s2p", "sh2", "g2",
    "qb", "kb", "ob", "nqw", "nkw",
    "cqb", "ckb", "cob", "cnqw", "cnkw",
    "n3w", "n3b", "b2",
    "mb0", "mb1", "mb2", "mb3",
]
_VBASE = {n: i * KC for i, n in enumerate(_VEC_NAMES)}
_VBASE["fb1"] = len(_VEC_NAMES) * KC
NV = len(_VEC_NAMES) * KC + FC




def build_bass(debug=False, stop_stage=99):
    nc = bacc.Bacc("TRN2", target_bir_lowering=False, debug=False,
                   num_devices=N_CORES)

    def din(name, shape, dt):
        return nc.dram_tensor(name, shape, dt, kind="ExternalInput").ap()

    io = dict(
        xT=din("xT", [DIM, TPC], F32),
        ctxT=din("ctxT", [DIM, CTXC], BF16),
        cosT=din("cosT", [HD, TPC], BF16),
        sinT=din("sinT", [HD, TPC], BF16),
        vecs_d=din("vecs", [128, NV], F32),
        rowb_d=din("rowb", [1, 2 * DIM], F32),
        wq=din("wq", [DIM, DIM], BF16),
        wk=din("wk", [DIM, DIM], BF16),
        wv=din("wv", [DIM, DIM], BF16),
        wo=din("wo", [DIM, DIM], BF16),
        cwq=din("cwq", [DIM, DIM], BF16),
        cwk=din("cwk", [DIM, DIM], BF16),
        cwv=din("cwv", [DIM, DIM], BF16),
        cwo=din("cwo", [DIM, DIM], BF16),
        w1=din("w1", [DIM, FFN], BF16),
        w2=din("w2", [FFN, DIM], BF16),
        moew=din("moew", [NE, DIM, DIM], BF16),
        wall_d=din("wall", [NE, TPC], F32),
        yT=nc.dram_tensor("yT", [DIM, TPC], F32, kind="ExternalOutput").ap(),
    )
    io["dbg"] = {}
    if debug:
        for nm, shp in [("h", [KC, 128, TPC]), ("k", [KC, 128, TPC]),
                        ("q", [KC, 128, TPC]), ("v", [CPB, 128, DIM]),
                        ("attnT", [KC, 128, TPC]), ("x2", [KC, 128, TPC]),
                        ("h3", [KC, 128, TPC]), ("cq", [KC, 128, TPC]),
                        ("cattnT", [KC, 128, TPC]),
                        ("x3", [KC, 128, TPC]), ("h2", [KC, 128, TPC]),
                        ("ff", [8, 128, TPC])]:
            io["dbg"][nm] = nc.dram_tensor("dbg_" + nm, shp, F32,
                                           kind="ExternalOutput").ap()

    with tile.TileContext(nc) as tc:
        _emit(nc, tc, io, stop_stage)
    nc.compile()
    return nc


def _emit(nc, tc, io, stop_stage):
    xT, ctxT, cosT, sinT = io["xT"], io["ctxT"], io["cosT"], io["sinT"]
    vecs_d, rowb_d = io["vecs_d"], io["rowb_d"]
    wq, wk, wv, wo = io["wq"], io["wk"], io["wv"], io["wo"]
    cwq, cwk, cwv, cwo = io["cwq"], io["cwk"], io["cwv"], io["cwo"]
    w1, w2, moew, wall_d = io["w1"], io["w2"], io["moew"], io["wall_d"]
    yT, dbg = io["yT"], io["dbg"]

    ctx = contextlib.ExitStack()
    const = ctx.enter_context(tc.tile_pool(name="const", bufs=1))
    xp = ctx.enter_context(tc.tile_pool(name="xp", bufs=6))       # transient f32
    hp = ctx.enter_context(tc.tile_pool(name="hp", bufs=13))      # h/h3/h2 bf16
    big = ctx.enter_context(tc.tile_pool(name="big", bufs=2))
    seq = ctx.enter_context(tc.tile_pool(name="seq", bufs=1))     # raw / ff slot
    lnp = ctx.enter_context(tc.tile_pool(name="lnp", bufs=4))
    wp = ctx.enter_context(tc.tile_pool(name="wp", bufs=3))
    wp2 = ctx.enter_context(tc.tile_pool(name="wp2", bufs=4))
    sm = ctx.enter_context(tc.tile_pool(name="sm", bufs=8))
    att = ctx.enter_context(tc.tile_pool(name="att", bufs=8))
    exq = ctx.enter_context(tc.tile_pool(name="exq", bufs=4))
    tmp = ctx.enter_context(tc.tile_pool(name="tmp", bufs=8))
    vcp = ctx.enter_context(tc.tile_pool(name="vcp", bufs=3))
    psA = ctx.enter_context(tc.tile_pool(name="psA", bufs=4, space="PSUM"))
    psN = ctx.enter_context(tc.tile_pool(name="psN", bufs=2, space="PSUM"))
    psB = ctx.enter_context(tc.tile_pool(name="psB", bufs=2, space="PSUM"))
    dram = ctx.enter_context(tc.tile_pool(name="dram", bufs=1, space="DRAM"))

    # ---------------- constants
    vecs = const.tile([128, NV], F32, name="vecs_s")
    nc.sync.dma_start(vecs[:], vecs_d[:])

    def vcol(name, c):
        i = _VBASE[name] + c
        return vecs[:, i:i + 1]

    rowb_bf = const.tile([1, 2 * DIM], BF16, name="rowb_bf")
    nc.gpsimd.dma_start(rowb_bf[:], rowb_d[:])      # casting DMA
    cos_t = const.tile([HD, TPC], BF16, name="cos_s")
    nc.sync.dma_start(cos_t[:], cosT[:])
    sin_t = const.tile([HD, TPC], BF16, name="sin_s")
    nc.sync.dma_start(sin_t[:], sinT[:])
    wallb = []
    for e in range(NE):
        wb = const.tile([128, TPC], F32, tag=f"wallb{e}", name=f"wallb{e}")
        bc = bass.AP(tensor=wall_d.tensor, offset=e * TPC, ap=[[0, 128], [1, TPC]])
        nc.sync.dma_start(wb[:], bc)
        wallb.append(wb)
    ones_c = const.tile([128, 1], BF16, name="ones_c")
    nc.vector.memset(ones_c[:], 1.0)
    ones_r = const.tile([1, 128], BF16, name="ones_r")
    nc.vector.memset(ones_r[:], 1.0)
    ones_rf = const.tile([1, 128], F32, name="ones_rf")
    nc.vector.memset(ones_rf[:], 1.0)
    eps_t = const.tile([1, 1], F32, name="eps_t")
    nc.vector.memset(eps_t[:], EPS)

    SCL = float(1.0 / np.sqrt(HD))

    # residual scratch in DRAM
    xs2 = dram.tile([DIM, TPC], F32, name="xs2")
    xs3 = dram.tile([DIM, TPC], F32, name="xs3")

    def load_x(src, c):
        xc = xp.tile([128, TPC], F32, tag="x", name="xc")
        nc.sync.dma_start(xc[:], src[ts(c, 128), :])
        return xc

    def dbg_dump(name, chunks):
        if name in dbg:
            d = dbg[name]
            for c, tl in enumerate(chunks):
                tf = tmp.tile([tl.shape[0], tl.shape[-1]], F32, tag="f32t",
                              name="dbgf")
                nc.vector.tensor_copy(tf[:], tl)
                nc.sync.dma_start(d[c, :tl.shape[0], :], tf[:])

    # ---------------- helpers
    def bcast_row(row_ap, n_tok):
        ps = psB.tile([128, n_tok], F32, tag="bc", name="bc_ps")
        nc.tensor.matmul(ps[:], ones_rf[:], row_ap, start=True, stop=True,
                         skip_group_check=True)
        return ps

    def layernorm(load, n_tok, sname=None, shname=None, wname=None,
                  bname=None, dump=None):
        """mean/var over partitions via ones-matmuls; returns bf16 chunks."""
        ps_s = psN.tile([1, n_tok], F32, tag="nsum", name="ln_ps_s")
        ps_q = psN.tile([1, n_tok], F32, tag="nsum", name="ln_ps_q")
        for c in range(KC):
            xc = load(c)
            xb = lnp.tile([128, n_tok], BF16, tag="lnxb", name="lnxb")
            nc.vector.tensor_copy(xb[:], xc[:])
            nc.tensor.matmul(ps_s[:], ones_c[:], xb[:], start=(c == 0),
                             stop=(c == KC - 1), skip_group_check=True)
            sq = lnp.tile([128, n_tok], BF16, tag="lnsq", name="lnsq")
            nc.vector.tensor_mul(sq[:], xb[:], xb[:])
            nc.tensor.matmul(ps_q[:], ones_c[:], sq[:], start=(c == 0),
                             stop=(c == KC - 1), skip_group_check=True)
        mean = sm.tile([1, n_tok], F32, tag="s", name="mean")
        nc.scalar.activation(mean[:], ps_s[:], ACTF.Copy, scale=1.0 / DIM)
        ex2 = sm.tile([1, n_tok], F32, tag="s", name="ex2")
        nc.scalar.activation(ex2[:], ps_q[:], ACTF.Copy, scale=1.0 / DIM)
        m2 = sm.tile([1, n_tok], F32, tag="s", name="m2")
        nc.vector.tensor_mul(m2[:], mean[:], mean[:])
        var = sm.tile([1, n_tok], F32, tag="s", name="var")
        nc.vector.tensor_sub(var[:], ex2[:], m2[:])
        std = sm.tile([1, n_tok], F32, tag="s", name="std")
        nc.scalar.activation(std[:], var[:], ACTF.Sqrt, bias=eps_t[:1, :])
        rstd = sm.tile([1, n_tok], F32, tag="s", name="rstd")
        nc.vector.reciprocal(rstd[:], std[:])
        mb_ps = bcast_row(mean[:], n_tok)
        rb_ps = bcast_row(rstd[:], n_tok)
        mb = tmp.tile([128, n_tok], F32, tag="f32t", name="mb")
        nc.vector.tensor_copy(mb[:], mb_ps[:])
        rb = tmp.tile([128, n_tok], F32, tag="f32t", name="rb")
        nc.vector.tensor_copy(rb[:], rb_ps[:])
        out = []
        for c in range(KC):
            xc = load(c)
            u = tmp.tile([128, n_tok], F32, tag="f32t", name="lnu")
            nc.vector.tensor_sub(u[:], xc[:], mb[:])
            u2 = tmp.tile([128, n_tok], F32, tag="f32t", name="lnu2")
            nc.vector.tensor_mul(u2[:], u[:], rb[:])
            o = hp.tile([128, n_tok], BF16, tag="h", name="h_t")
            if sname is not None:
                nc.scalar.activation(o[:], u2[:], ACTF.Identity,
                                     bias=vcol(shname, c), scale=vcol(sname, c))
            else:
                nc.scalar.activation(o[:], u2[:], ACTF.Identity,
                                     bias=vcol(bname, c), scale=vcol(wname, c))
            out.append(o)
        if dump:
            dbg_dump(dump, [t[:] for t in out])
        return out

    def proj_dim_major(h_chunks, w_d, n_tok, consumer):
        for o in range(KC):
            wt = wp.tile([128, KC, 128], BF16, tag="w", name="wt")
            nc.sync.dma_start(
                wt[:], w_d[:, ts(o, 128)].rearrange("(k p) o -> p k o", p=128))
            ps = psA.tile([128, n_tok], F32, tag="mm", name="proj_ps")
            for k in range(KC):
                nc.tensor.matmul(ps[:], wt[:, k, :], h_chunks[k],
                                 start=(k == 0), stop=(k == KC - 1),
                                 skip_group_check=True)
            consumer(o, ps)

    def rms_apply(raw_chunks, wname, n_tok, out_tile_fn):
        ps_q = psN.tile([1, n_tok], F32, tag="nsum", name="rms_ps")
        for c in range(KC):
            s = lnp.tile([128, n_tok], BF16, tag="lnsq", name="rmsq")
            nc.vector.tensor_mul(s[:], raw_chunks[c], raw_chunks[c])
            nc.tensor.matmul(ps_q[:], ones_c[:], s[:], start=(c == 0),
                             stop=(c == KC - 1), skip_group_check=True)
        ms = sm.tile([1, n_tok], F32, tag="s", name="rms_ms")
        nc.scalar.activation(ms[:], ps_q[:], ACTF.Sqrt, bias=eps_t[:1, :],
                             scale=1.0 / DIM)
        rstd = sm.tile([1, n_tok], F32, tag="s", name="rms_r")
        nc.vector.reciprocal(rstd[:], ms[:])
        rb_ps = bcast_row(rstd[:], n_tok)
        rb = tmp.tile([128, n_tok], F32, tag="f32t", name="rms_rb")
        nc.vector.tensor_copy(rb[:], rb_ps[:])
        outs = []
        for c in range(KC):
            u = tmp.tile([128, n_tok], F32, tag="f32t", name="rms_u")
            nc.vector.tensor_mul(u[:], raw_chunks[c], rb[:])
            o = out_tile_fn(c)
            nc.scalar.activation(o, u[:], ACTF.Identity, scale=vcol(wname, c))
            outs.append(o)
        return outs

    def rope_inplace(chunks, n_tok):
        # q_rot = q * cos2 + swap(q) * sin2s, with cos2=[cos;cos],
        # sin2s=[-sin;sin] prepared on host and swap done via SBUF DMA.
        for c in range(KC):
            q = chunks[c]
            qs = tmp.tile([128, n_tok], BF16, tag="bft", name="rpswap")
            nc.sync.dma_start(qs[0:64, :], q[64:128, :])
            nc.sync.dma_start(qs[64:128, :], q[0:64, :])
            t1 = tmp.tile([128, n_tok], BF16, tag="bft", name="rp1")
            nc.vector.tensor_mul(t1[:], q, cos_t[:, :n_tok])
            t2 = tmp.tile([128, n_tok], BF16, tag="bft", name="rp2")
            nc.vector.tensor_mul(t2[:], qs[:], sin_t[:, :n_tok])
            nc.vector.tensor_add(q, t1[:], t2[:])

    def tokmajor_proj(h_chunks, w_d, bias_off, out_cb, n_tok):
        """out[tok128, dim] = h.T @ W + row-bias; out_cb(tw, ob, psum)."""
        n_tw = n_tok // 128
        for ob in range(3):
            pss = [psA.tile([128, 512], F32, tag="mm", name=f"vps{i}")
                   for i in range(n_tw)]
            for k in range(KC):
                wvb = wp.tile([128, 512], BF16, tag="wvb", name="wvb")
                nc.sync.dma_start(wvb[:], w_d[ts(k, 128), ds(ob * 512, 512)])
                for tw in range(n_tw):
                    nc.tensor.matmul(pss[tw][:],
                                     h_chunks[k][:, ts(tw, 128)], wvb[:],
                                     start=(k == 0), stop=False,
                                     skip_group_check=True)
            for tw in range(n_tw):
                nc.tensor.matmul(pss[tw][:], ones_r[:],
                                 rowb_bf[:, ds(bias_off + ob * 512, 512)],
                                 start=False, stop=True, skip_group_check=True)
                out_cb(tw, ob, pss[tw])

    # ================= stage 1: ln1 + modulation -> h
    h = layernorm(lambda c: load_x(xT, c), TPC, sname="s1p", shname="sh1",
                  dump="h")
    if stop_stage <= 1:
        for c in range(KC):
            hc = tmp.tile([128, TPC], F32, tag="f32t", name="hf")
            nc.vector.tensor_copy(hc[:], h[c][:])
            nc.sync.dma_start(yT[ts(c, 128), :], hc[:])
        ctx.close()
        return

    # ================= stage 2: k, v, ctx-k, ctx-v
    k_raw = seq.tile([128, KC, TPC], BF16, tag="seq", name="k_raw")

    def k_cons(o, ps):
        nc.scalar.activation(k_raw[:, o, :], ps[:], ACTF.Identity,
                             bias=vcol("kb", o))

    proj_dim_major([h[c][:] for c in range(KC)], wk, TPC, k_cons)
    kT = big.tile([128, KC, TPC], BF16, tag="kv", name="kT")
    rms_apply([k_raw[:, c, :] for c in range(KC)], "nkw", TPC,
              lambda c: kT[:, c, :])
    rope_inplace([kT[:, c, :] for c in range(KC)], TPC)
    dbg_dump("k", [kT[:, c, :] for c in range(KC)])

    # bounce buffers (DRAM)
    inb = dram.tile([GATHER_ELEMS], BF16, name="inb")
    gout = dram.tile([CPB, GATHER_ELEMS], BF16, name="gout")

    # v: token-major, psum -> small bf16 tile -> straight to bounce
    def v_cons(tw, ob, ps):
        vt = vcp.tile([128, 512], BF16, tag="vc", name="vtmp")
        nc.vector.tensor_copy(vt[:], ps[:])
        nc.sync.dma_start(
            bass.AP(tensor=inb.tensor,
                    offset=inb.offset + SZ_K + tw * 128 * DIM + ob * 512,
                    ap=[[DIM, 128], [1, 512]]),
            vt[:])
        if "v" in dbg:
            tf = tmp.tile([128, 512], F32, tag="f32t", name="dbgv")
            nc.vector.tensor_copy(tf[:], ps[:])
            nc.sync.dma_start(dbg["v"][tw, :, ds(ob * 512, 512)], tf[:])

    tokmajor_proj([h[c][:] for c in range(KC)], wv, 0, v_cons, TPC)

    ctx_ch = []
    for c in range(KC):
        cc = const.tile([128, CTXC], BF16, tag=f"ctx{c}", name=f"ctxc{c}")
        nc.sync.dma_start(cc[:], ctxT[ts(c, 128), :])
        ctx_ch.append(cc)
    ck_raw = big.tile([128, KC, CTXC], BF16, tag="craw", name="ck_raw")

    def ck_cons(o, ps):
        nc.scalar.activation(ck_raw[:, o, :], ps[:], ACTF.Identity,
                             bias=vcol("ckb", o))

    proj_dim_major([ctx_ch[c][:] for c in range(KC)], cwk, CTXC, ck_cons)
    ckT = big.tile([128, KC, CTXC], BF16, tag="ckv", name="ckT")
    rms_apply([ck_raw[:, c, :] for c in range(KC)], "cnkw", CTXC,
              lambda c: ckT[:, c, :])

    def cv_cons(tw, ob, ps):
        vt = vcp.tile([128, 512], BF16, tag="vc", name="cvtmp")
        nc.vector.tensor_copy(vt[:], ps[:])
        nc.sync.dma_start(
            bass.AP(tensor=inb.tensor,
                    offset=inb.offset + SZ_K + SZ_V + SZ_CK + ob * 512,
                    ap=[[DIM, 128], [1, 512]]),
            vt[:])

    tokmajor_proj([ctx_ch[c][:] for c in range(KC)], cwv, DIM, cv_cons, CTXC)

    # ================= stage 3: bounce K/cK + AllGather
    off = 0
    for c in range(KC):
        nc.sync.dma_start(
            inb[ds(off, 128 * TPC)].rearrange("(p t) -> p t", p=128),
            kT[:, c, :])
        off += 128 * TPC
    off = SZ_K + SZ_V
    for c in range(KC):
        nc.sync.dma_start(
            inb[ds(off, 128 * CTXC)].rearrange("(p t) -> p t", p=128),
            ckT[:, c, :])
        off += 128 * CTXC
    groups = [list(range(CPB)), list(range(CPB, 2 * CPB))]
    nc.gpsimd.collective_compute(
        "AllGather", ALU.bypass, replica_groups=groups,
        ins=[inb.opt()], outs=[gout.opt()])

    gK = gout[:, ds(0, SZ_K)].rearrange("s (c p t) -> s c p t", c=KC, p=128)
    gV = gout[:, ds(SZ_K, SZ_V)].rearrange("s (w p d) -> s w p d", w=CPB, p=128)
    gCK = gout[:, ds(SZ_K + SZ_V, SZ_CK)].rearrange(
        "s (c p t) -> s c p t", c=KC, p=128)
    gCV = gout[:, ds(SZ_K + SZ_V + SZ_CK, SZ_CV)].rearrange(
        "s (p d) -> s p d", p=128)

    # ================= stage 4: q (overlaps gather)
    q_raw = seq.tile([128, KC, TPC], BF16, tag="seq", name="q_raw")

    def q_cons(o, ps):
        nc.scalar.activation(q_raw[:, o, :], ps[:], ACTF.Identity,
                             bias=vcol("qb", o))

    proj_dim_major([h[c][:] for c in range(KC)], wq, TPC, q_cons)
    qT = big.tile([128, KC, TPC], BF16, tag="kv", name="qT")
    rms_apply([q_raw[:, c, :] for c in range(KC)], "nqw", TPC,
              lambda c: qT[:, c, :])
    rope_inplace([qT[:, c, :] for c in range(KC)], TPC)
    dbg_dump("q", [qT[:, c, :] for c in range(KC)])
    if stop_stage <= 4:
        for c in range(KC):
            hc = tmp.tile([128, TPC], F32, tag="f32t", name="qf")
            nc.vector.tensor_copy(hc[:], qT[:, c, :])
            nc.sync.dma_start(yT[ts(c, 128), :], hc[:])
        ctx.close()
        return

    # ================= stage 5/7: attention
    def attention(q_big, n_kch, kt_src, vt_src, out_big):
        for hh in range(NH):
            aps = psA.tile([128, TPC], F32, tag="mm", name="aps")
            dps = psN.tile([1, TPC], F32, tag="nsum", name="dps")
            for ci in range(n_kch):
                kt = att.tile([128, 128], BF16, tag="kv128", name="kt")
                nc.sync.dma_start(kt[:], kt_src(ci, hh))
                sps = psA.tile([128, TPC], F32, tag="mm", name="sps")
                nc.tensor.matmul(sps[:], kt[:], q_big[:, hh, :],
                                 start=True, stop=True, skip_group_check=True)
                ex = exq.tile([128, TPC], BF16, tag="ex", name="ex")
                nc.scalar.activation(ex[:], sps[:], ACTF.Exp, scale=SCL)
                vt = att.tile([128, 128], BF16, tag="kv128", name="vt")
                nc.sync.dma_start(vt[:], vt_src(ci, hh))
                nc.tensor.matmul(aps[:], vt[:], ex[:], start=(ci == 0),
                                 stop=(ci == n_kch - 1), skip_group_check=True)
                nc.tensor.matmul(dps[:], ones_c[:], ex[:], start=(ci == 0),
                                 stop=(ci == n_kch - 1), skip_group_check=True)
            rec = sm.tile([1, TPC], F32, tag="s", name="rec")
            nc.vector.reciprocal(rec[:], dps[:])
            rb_ps = bcast_row(rec[:], TPC)
            rb = tmp.tile([128, TPC], F32, tag="f32t", name="arb")
            nc.vector.tensor_copy(rb[:], rb_ps[:])
            nc.vector.tensor_mul(out_big[:, hh, :], aps[:], rb[:])

    attnT = big.tile([128, KC, TPC], BF16, tag="kv", name="attnT")
    attention(qT, CPB * 4,
              lambda ci, hh: gK[ci // 4, hh, :, ts(ci % 4, 128)],
              lambda ci, hh: gV[ci // 4, ci % 4, :, ts(hh, 128)],
              attnT)
    dbg_dump("attnT", [attnT[:, c, :] for c in range(KC)])

    # ================= stage 6: o-proj + gated residual -> x2 (dram)
    def o_cons(o, ps):
        t1 = tmp.tile([128, TPC], F32, tag="f32t", name="ot1")
        nc.vector.tensor_scalar(t1[:], ps[:], vcol("ob", o), vcol("g1", o),
                                ALU.add, ALU.mult)
        xc = load_x(xT, o)
        x2 = xp.tile([128, TPC], F32, tag="x", name="x2c")
        nc.vector.tensor_add(x2[:], t1[:], xc[:])
        nc.sync.dma_start(xs2[ts(o, 128), :], x2[:])

    proj_dim_major([attnT[:, c, :] for c in range(KC)], wo, TPC, o_cons)
    if "x2" in dbg:
        for c in range(KC):
            xc = load_x(xs2, c)
            nc.sync.dma_start(dbg["x2"][c, :, :], xc[:])
    if stop_stage <= 6:
        for c in range(KC):
            xc = load_x(xs2, c)
            nc.sync.dma_start(yT[ts(c, 128), :], xc[:])
        ctx.close()
        return

    # ================= stage 7: norm3 -> h3, cq, cross attention
    h3 = layernorm(lambda c: load_x(xs2, c), TPC, wname="n3w", bname="n3b",
                   dump="h3")
    cq_raw = seq.tile([128, KC, TPC], BF16, tag="seq", name="cq_raw")

    def cq_cons(o, ps):
        nc.scalar.activation(cq_raw[:, o, :], ps[:], ACTF.Identity,
                             bias=vcol("cqb", o))

    proj_dim_major([h3[c][:] for c in range(KC)], cwq, TPC, cq_cons)
    cqT = big.tile([128, KC, TPC], BF16, tag="kv", name="cqT")
    rms_apply([cq_raw[:, c, :] for c in range(KC)], "cnqw", TPC,
              lambda c: cqT[:, c, :])
    dbg_dump("cq", [cqT[:, c, :] for c in range(KC)])

    cattnT = big.tile([128, KC, TPC], BF16, tag="kv", name="cattnT")
    attention(cqT, CPB,
              lambda ci, hh: gCK[ci, hh, :, :],
              lambda ci, hh: gCV[ci, :, ts(hh, 128)],
              cattnT)
    dbg_dump("cattnT", [cattnT[:, c, :] for c in range(KC)])

    # ================= stage 8: ca o-proj + residual -> x3 (dram)
    def co_cons(o, ps):
        t1 = tmp.tile([128, TPC], F32, tag="f32t", name="cot1")
        nc.vector.tensor_scalar_add(t1[:], ps[:], vcol("cob", o))
        xc = load_x(xs2, o)
        x3 = xp.tile([128, TPC], F32, tag="x", name="x3c")
        nc.vector.tensor_add(x3[:], t1[:], xc[:])
        nc.sync.dma_start(xs3[ts(o, 128), :], x3[:])

    proj_dim_major([cattnT[:, c, :] for c in range(KC)], cwo, TPC, co_cons)
    if "x3" in dbg:
        for c in range(KC):
            xc = load_x(xs3, c)
            nc.sync.dma_start(dbg["x3"][c, :, :], xc[:])
    if stop_stage <= 8:
        for c in range(KC):
            xc = load_x(xs3, c)
            nc.sync.dma_start(yT[ts(c, 128), :], xc[:])
        ctx.close()
        return

    # ================= stage 9: ln2 -> h2
    h2 = layernorm(lambda c: load_x(xs3, c), TPC, sname="s2p", shname="sh2",
                   dump="h2")

    # ================= stage 10: ffn + moe in two token halves
    for th in range(2):
        tsl = ds(th * HT, HT)
        ff = seq.tile([128, FC, HT], BF16, tag="seq", name=f"ff{th}")
        for o in range(FC):
            wt = wp.tile([128, KC, 128], BF16, tag="w", name="w1t")
            nc.sync.dma_start(
                wt[:], w1[:, ts(o, 128)].rearrange("(k p) o -> p k o", p=128))
            ps = psA.tile([128, HT], F32, tag="mm", name="ffps")
            for k in range(KC):
                nc.tensor.matmul(ps[:], wt[:, k, :], h2[k][:, tsl],
                                 start=(k == 0), stop=(k == KC - 1),
                                 skip_group_check=True)
            nc.scalar.activation(ff[:, o, :], ps[:], ACTF.Gelu_apprx_tanh,
                                 bias=vcol("fb1", o))
            if "ff" in dbg and th == 0 and o < 8:
                tf = tmp.tile([128, HT], F32, tag="f32t", name="dbgff")
                nc.vector.tensor_copy(tf[:], ff[:, o, :])
                nc.sync.dma_start(dbg["ff"][o, :, ds(0, HT)], tf[:])

        for o in range(KC):
            ps = psA.tile([128, HT], F32, tag="mm", name="w2ps")
            for k in range(FC):
                w2b = wp2.tile([128, 128], BF16, tag="w2b", name="w2b")
                nc.sync.dma_start(w2b[:], w2[ts(k, 128), ts(o, 128)])
                nc.tensor.matmul(ps[:], w2b[:], ff[:, k, :],
                                 start=(k == 0), stop=(k == FC - 1),
                                 skip_group_check=True)
            acc = tmp.tile([128, HT], F32, tag="f32t", name="macc")
            nc.vector.tensor_scalar_add(acc[:], ps[:], vcol("b2", o))
            for e in range(NE):
                met = wp.tile([128, KC, 128], BF16, tag="w", name="moet")
                nc.sync.dma_start(
                    met[:],
                    moew[e, :, ts(o, 128)].rearrange("(k p) o -> p k o", p=128))
                pse = psA.tile([128, HT], F32, tag="mm", name="pse")
                for k in range(KC):
                    nc.tensor.matmul(pse[:], met[:, k, :], h2[k][:, tsl],
                                     start=(k == 0), stop=(k == KC - 1),
                                     skip_group_check=True)
                te = tmp.tile([128, HT], F32, tag="f32t", name="te")
                nc.vector.scalar_tensor_tensor(te[:], pse[:], vcol(f"mb{e}", o),
                                               wallb[e][:, tsl],
                                               ALU.add, ALU.mult)
                acc2 = tmp.tile([128, HT], F32, tag="f32t", name="macc2")
                nc.vector.tensor_add(acc2[:], acc[:], te[:])
                acc = acc2
            t1 = tmp.tile([128, HT], F32, tag="f32t", name="yt1")
            nc.vector.tensor_scalar_mul(t1[:], acc[:], vcol("g2", o))
            xc = load_x(xs3, o)
            yc = tmp.tile([128, HT], F32, tag="f32t", name="yc")
            nc.vector.tensor_add(yc[:], t1[:], xc[:, tsl])
            nc.sync.dma_start(yT[ts(o, 128), tsl], yc[:])

    ctx.close()


# -------------------------------------------------------------- host prep
def _rope_perm():
    p = np.arange(DIM).reshape(NH, HD)
    return np.concatenate([p[:, 0::2], p[:, 1::2]], axis=1).reshape(-1)


def prep_inputs(inputs):
    f = lambda a: np.asarray(a, dtype=np.float32)
    x = f(inputs["x"])
    context = f(inputs["context"])
    t_mod = f(inputs["t_mod"])
    freqs_cos = f(inputs["freqs_cos"])
    freqs_sin = f(inputs["freqs_sin"])
    ew = f(inputs["expert_weights"])
    idx = np.asarray(inputs["top_k_indices"])
    modulation = f(inputs["modulation"])

    perm = _rope_perm()

    def wT(a):
        return np.ascontiguousarray(f(a).T).astype(bfnp)

    wq_h = np.ascontiguousarray(f(inputs["sa_q_w"])[perm].T).astype(bfnp)
    wk_h = np.ascontiguousarray(f(inputs["sa_k_w"])[perm].T).astype(bfnp)
    wv_h = wT(inputs["sa_v_w"])
    wo_h = wT(inputs["sa_o_w"])
    cwq_h = wT(inputs["ca_q_w"])
    cwk_h = wT(inputs["ca_k_w"])
    cwv_h = wT(inputs["ca_v_w"])
    cwo_h = wT(inputs["ca_o_w"])
    w1_h = wT(inputs["ffn_w1"])
    w2_h = wT(inputs["ffn_w2"])
    moew_h = np.ascontiguousarray(f(inputs["moe_w"]).transpose(0, 2, 1)).astype(bfnp)
    moeb = f(inputs["moe_b"])

    mod = modulation + t_mod
    cosA = np.concatenate([freqs_cos.T, freqs_cos.T], 0).astype(bfnp)
    sinA = np.concatenate([-freqs_sin.T, freqs_sin.T], 0).astype(bfnp)

    in_maps, metas = [], []
    for c in range(N_CORES):
        b, i = c // CPB, c % CPB
        tok = slice(i * TPC, (i + 1) * TPC)
        ctok = slice(i * CTXC, (i + 1) * CTXC)
        vecs = np.zeros((128, NV), np.float32)

        def setv(name, arr):
            n = len(arr) // 128
            vecs[:, _VBASE[name]:_VBASE[name] + n] = arr.reshape(n, 128).T

        m = mod[b]
        setv("s1p", 1.0 + m[1]); setv("sh1", m[0]); setv("g1", m[2])
        setv("s2p", 1.0 + m[4]); setv("sh2", m[3]); setv("g2", m[5])
        setv("qb", f(inputs["sa_q_b"])[perm])
        setv("kb", f(inputs["sa_k_b"])[perm])
        setv("ob", f(inputs["sa_o_b"]))
        setv("nqw", f(inputs["sa_nq_w"])[perm])
        setv("nkw", f(inputs["sa_nk_w"])[perm])
        setv("cqb", f(inputs["ca_q_b"])); setv("ckb", f(inputs["ca_k_b"]))
        setv("cob", f(inputs["ca_o_b"]))
        setv("cnqw", f(inputs["ca_nq_w"])); setv("cnkw", f(inputs["ca_nk_w"]))
        setv("n3w", f(inputs["norm3_w"])); setv("n3b", f(inputs["norm3_b"]))
        setv("b2", f(inputs["ffn_b2"])); setv("fb1", f(inputs["ffn_b1"]))
        for e in range(NE):
            setv(f"mb{e}", moeb[e])

        rowb = np.concatenate([f(inputs["sa_v_b"]), f(inputs["ca_v_b"])])[None, :]

        wall = np.zeros((NE, TPC), np.float32)
        iw = idx[b, tok]
        eww = ew[b, tok]
        for kk in range(TOPK):
            np.add.at(wall, (iw[:, kk], np.arange(TPC)), eww[:, kk])

        in_maps.append({
            "xT": np.ascontiguousarray(x[b, tok].T),
            "ctxT": np.ascontiguousarray(context[b, ctok].T).astype(bfnp),
            "cosT": np.ascontiguousarray(cosA[:, tok]),
            "sinT": np.ascontiguousarray(sinA[:, tok]),
            "vecs": vecs,
            "rowb": rowb.astype(np.float32),
            "wq": wq_h, "wk": wk_h, "wv": wv_h, "wo": wo_h,
            "cwq": cwq_h, "cwk": cwk_h, "cwv": cwv_h, "cwo": cwo_h,
            "w1": w1_h, "w2": w2_h, "moew": moew_h,
            "wall": wall.astype(np.float32),
        })
        metas.append((b, i))
    return in_maps, metas


_NC_CACHE = {}


def get_nc(debug=False, stop_stage=99):
    key = (bool(debug), stop_stage)
    if key not in _NC_CACHE:
        _NC_CACHE[key] = build_bass(debug=debug, stop_stage=stop_stage)
    return _NC_CACHE[key]


def run(in_maps, debug=False, stop_stage=99):
    nc = get_nc(debug=debug, stop_stage=stop_stage)
    return bass_utils.run_bass_kernel_spmd(
        nc, in_maps, core_ids=list(range(N_CORES)), trace=False)


def kernel(**inputs):
    in_maps, metas = prep_inputs(inputs)
    res = run(in_maps, debug=bool(int(os.environ.get("BASSDIT_DEBUG", "0"))),
              stop_stage=int(os.environ.get("BASSDIT_STOP", "99")))
    out = np.zeros((B, S, DIM), np.float32)
    for c in range(N_CORES):
        b, i = metas[c]
        out[b, i * TPC:(i + 1) * TPC] = np.asarray(
            res.results[c]["yT"], dtype=np.float32).T
    kernel.last_results = res
    return out
